# revision 22
# baseline (speedup 1.0000x reference)
"""BFFN (linear-attention style gated FFN) Trainium2 Bass kernel, 8 NeuronCores.

Reference computation (all fp32, B=4, N=4096, D=E=1024):
    query = (x_real @ Wqr) * (x_imag @ Wqi)        # [b, n, e]
    key   = x_real @ Wk                             # [b, n, d]
    value = x_imag @ Wv                             # [b, n, e]
    kv    = einsum('bnd,bne->bde', key, value)      # [b, d, e]
    out   = einsum('bnd,bde->bne', query, kv)       # [b, n, e]

Algebraic restructure: kv = Wk^T @ (xr^T @ xi) @ Wv.  With S = xr^T @ xi
(the only sequence reduction), the kv path costs N*D*D + 2*D*D*E FLOPs
instead of 3*N*D*E, and S comes straight from x in natural layout.

Sharding: 8 cores = 4 batches x 2 sequence-halves.  Each pair AllReduces
its partial S (two bf16 1MB halves, flat layout for fat DMA descriptors).
The kv chain is split across the pair: the host slices Wk per core, so
core h computes UT = S^T Wk[:, h-half] then its 512 kv rows, and a 1MB
AllGather assembles the full kv.

All operands are pre-cast to bf16 on the HOST (the kernel computed in
bf16 anyway - identical numerics, half the HBM traffic, no staging or
cast stage on-chip).  HBM bandwidth is split roughly evenly per active
DMA queue row, so x streams round-robin across sync/scalar/gpsimd;
bounce/prefetch/output traffic is spread across the queues to keep the
collective chain off the critical path.

Per-core schedule:
  stream x:  small bf16 chunks straight into x_nat across all three
             DMA paths.  S-left accumulates for d-tiles 0-5 (PSUM
             banks 0-5) while every arriving tile is transposed via
             regular matmuls lhsT=x_tile, rhs=identity (banks 6-7) -
             pipelined N=128 matmuls (~81ns), not is_transpose mode
             (~219ns).  From nt=12 (transposes done) the d 6-7
             leftover backlog accumulates in the freed banks.
  post:      bounce + AllReduce S-left; chunk-3 transposes; S-right
             (8 banks); bounce + AllReduce S-right.
  query:     chunked qT = (Wq^T xT) gating with UT / kv / kv-AllGather
             interleaved between chunks, then the out chunks.
All matmuls bf16 operands, fp32 PSUM accumulation.  SBUF is fully
budgeted (~196KB/partition): one 8KB-slot pool chains xT chunks ->
qt chunks / kv tiles; the x_nat slots are reused by the reduced-S
prefetch.
"""
import numpy as np
import ml_dtypes

import concourse.bass as bass
import concourse.mybir as mybir
import concourse.tile as tile
from concourse import bacc
from concourse.bass import ts, ds
from concourse.bass_utils import run_bass_kernel_spmd
from concourse.masks import make_identity

F32 = mybir.dt.float32
BF16 = mybir.dt.bfloat16
NP_BF16 = ml_dtypes.bfloat16

B, N, D, E = 4, 4096, 1024, 1024
N_CORES = 8
NL = N // 2          # 2048 rows (sequence) per core
P = 128
NT = NL // P         # 16 n-tiles
DT = D // P          # 8 d tiles
ET = E // P          # 8 e tiles
FD = 512             # matmul moving free dim / PSUM bank
NCH = NL // FD       # 4 n-chunks of 512
KH = FD              # per-core kv row count (Wk half)
NSL = 6              # S-left d-tiles accumulated while streaming

REPLICA_GROUPS = [[0, 1], [2, 3], [4, 5], [6, 7]]


def build_bass():
    nc = bacc.Bacc("TRN2", target_bir_lowering=False, debug=False,
                   num_devices=N_CORES)

    xr = nc.dram_tensor("xr", [NL, D], BF16, kind="ExternalInput").ap()
    xi = nc.dram_tensor("xi", [NL, D], BF16, kind="ExternalInput").ap()
    wqr = nc.dram_tensor("wqr", [D, E], BF16, kind="ExternalInput").ap()
    wqi = nc.dram_tensor("wqi", [D, E], BF16, kind="ExternalInput").ap()
    wkh = nc.dram_tensor("wkh", [D, KH], BF16, kind="ExternalInput").ap()
    wv = nc.dram_tensor("wv", [D, E], BF16, kind="ExternalInput").ap()
    out = nc.dram_tensor("out", [NL, E], F32, kind="ExternalOutput").ap()

    # Flat bounce buffers: SBUF tile rows land contiguously per partition.
    sL_in = nc.dram_tensor("sL_in", [P, DT * FD], BF16, kind="Internal").ap()
    sR_in = nc.dram_tensor("sR_in", [P, DT * FD], BF16, kind="Internal").ap()
    sL_out = nc.dram_tensor("sL_out", [P, DT * FD], BF16,
                            kind="Internal").ap()
    sR_out = nc.dram_tensor("sR_out", [P, DT * FD], BF16,
                            kind="Internal").ap()
    kv_in = nc.dram_tensor("kv_in", [P, 4 * E], BF16, kind="Internal").ap()
    kv_out = nc.dram_tensor("kv_out", [2, P, 4 * E], BF16,
                            kind="Internal").ap()

    def as_tiles(w):  # [1024, n] DRAM view -> [128, t, n] partition-major
        return w.rearrange("(t p) n -> p t n", p=P)

    with tile.TileContext(nc) as tc:
        with (
            tc.tile_pool(name="q8", bufs=9) as q8_pool,      # xT/qt/kv 8KB
            tc.tile_pool(name="b32", bufs=2) as b32_pool,    # x_nat -> s_sb
            tc.tile_pool(name="wq", bufs=1) as wq_pool,
            tc.tile_pool(name="wkv", bufs=2) as wkv_pool,    # wk -> wv_B
            tc.tile_pool(name="ut", bufs=1) as ut_pool,
            tc.tile_pool(name="sst", bufs=2) as sst_pool,    # bounce staging
            tc.tile_pool(name="ost", bufs=3) as out_pool,
            tc.tile_pool(name="cst", bufs=1) as cst_pool,
            tc.tile_pool(name="ps", bufs=8, space="PSUM") as ps_pool,
        ):
            ident = cst_pool.tile([P, P], BF16, tag="id", name="ident")
            make_identity(nc, ident)

            xr_nat = b32_pool.tile([P, NT, D], BF16, tag="b32", name="xr_nat")
            xi_nat = b32_pool.tile([P, NT, D], BF16, tag="b32", name="xi_nat")

            # stream x straight into x_nat, round-robined across all
            # three DMA paths (HBM bandwidth is split roughly evenly per
            # active queue row, so balance matters more than queue choice);
            # 1-tile lead chunks get the PE started ~5us sooner
            spans = [(0, 1), (1, 1), (2, 1)] + [(t, 2) for t in range(3, NT - 1, 2)] + [(NT - 1, 1)]
            chunks = []
            for lo, n in spans:
                chunks.append(("r", xr_nat, xr, lo, n))
                chunks.append(("i", xi_nat, xi, lo, n))
            queues = [nc.sync.dma_start, nc.scalar.dma_start,
                      nc.gpsimd.dma_start]
            for k, (_, x_nat, x_dram, lo, n) in enumerate(chunks):
                queues[k % 3](
                    x_nat[:, lo:lo + n, :],
                    x_dram[lo * P:(lo + n) * P, :]
                    .rearrange("(t p) d -> p t d", p=P))

            # xT chunks: [d-part, d-tile, n-in-chunk] per x tensor
            xT = {}
            for c in range(NCH):
                xT[("r", c)] = q8_pool.tile([P, DT, FD], BF16, tag="q8",
                                            name=f"xrT{c}")
                xT[("i", c)] = q8_pool.tile([P, DT, FD], BF16, tag="q8",
                                            name=f"xiT{c}")

            def transpose_tile(kind, x_nat, nt):
                c, j = divmod(nt, 4)
                for half in range(2):
                    ps_t = ps_pool.tile([P, 4, P], F32, tag="ps", name="ps_t")
                    for dj in range(4):
                        d = 4 * half + dj
                        nc.tensor.matmul(
                            ps_t[:, dj, :], x_nat[:, nt, ts(d, P)],
                            ident[:], start=True, stop=True,
                        )
                    nc.vector.tensor_copy(
                        xT[(kind, c)][:, ts(half, 4), ts(j, P)], ps_t[:])

            # ---- stream: S-left (d 0-5) + transposes (+ d 6-7 backlog) ----
            ps_s = [ps_pool.tile([P, FD], F32, tag="ps", name="ps_s")
                    for _ in range(NSL)]
            ps_s2 = []
            for nt in range(NT):
                for d in range(NSL):
                    nc.tensor.matmul(
                        ps_s[d][:], xr_nat[:, nt, ts(d, P)],
                        xi_nat[:, nt, :FD],
                        start=(nt == 0), stop=(nt == NT - 1),
                    )
                if nt < 12:
                    transpose_tile("r", xr_nat, nt)
                    transpose_tile("i", xi_nat, nt)
                else:
                    # banks 6-7 free from nt=12: drain the d 6-7 backlog
                    if nt == 12:
                        ps_s2 = [ps_pool.tile([P, FD], F32, tag="ps",
                                              name="ps_s2")
                                 for _ in range(DT - NSL)]
                        backlog = list(range(13))
                    else:
                        backlog = [nt]
                    for bnt in backlog:
                        for i, d in enumerate(range(NSL, DT)):
                            nc.tensor.matmul(
                                ps_s2[i][:], xr_nat[:, bnt, ts(d, P)],
                                xi_nat[:, bnt, :FD],
                                start=(bnt == 0), stop=(bnt == NT - 1),
                            )

            # ---- S-left evac + bounce + AllReduce (frees the banks the
            # chunk-3 transposes below need - order matters) ----
            def bounce_s(ps_banks, dram_half, queue):
                view = dram_half.rearrange("p (h t n) -> p h t n", h=2, t=4)
                for h in range(2):
                    s_st = sst_pool.tile([P, 4, FD], BF16, tag="sst",
                                         name="s_st")
                    for t in range(4):
                        nc.vector.tensor_copy(s_st[:, t, :],
                                              ps_banks[4 * h + t][:])
                    queue(view[:, h], s_st[:])

            bounce_s(ps_s + ps_s2, sL_in, nc.sync.dma_start)
            nc.gpsimd.collective_compute(
                "AllReduce", mybir.AluOpType.add,
                replica_groups=REPLICA_GROUPS,
                ins=[sL_in.opt()], outs=[sL_out.opt()],
            )

            # transposes of chunk 3 (nt 12-15)
            for nt in range(12, 16):
                transpose_tile("r", xr_nat, nt)
                transpose_tile("i", xi_nat, nt)

            # weights: single bf16 DMAs on the queues behind the x stream
            wqr_sb = wq_pool.tile([P, DT, E], BF16, tag="wqr", name="wqr_sb")
            nc.sync.dma_start(wqr_sb[:], as_tiles(wqr))
            wqi_sb = wq_pool.tile([P, DT, E], BF16, tag="wqi", name="wqi_sb")
            nc.scalar.dma_start(wqi_sb[:], as_tiles(wqi))
            wk_sb = wkv_pool.tile([P, DT, KH], BF16, tag="wkv", name="wk_sb")
            nc.sync.dma_start(wk_sb[:], as_tiles(wkh))
            wv_A = wkv_pool.tile([P, DT, FD], BF16, tag="wkv", name="wv_A")
            nc.scalar.dma_start(wv_A[:], as_tiles(wv)[:, :, :FD])

            # ---- S-right from resident x (8 banks) ----
            ps_s3 = [ps_pool.tile([P, FD], F32, tag="ps", name="ps_s3")
                     for _ in range(DT)]
            for nt in range(NT):
                for d in range(DT):
                    nc.tensor.matmul(
                        ps_s3[d][:], xr_nat[:, nt, ts(d, P)],
                        xi_nat[:, nt, FD:],
                        start=(nt == 0), stop=(nt == NT - 1),
                    )
            bounce_s(ps_s3, sR_in, nc.scalar.dma_start)
            nc.gpsimd.collective_compute(
                "AllReduce", mybir.AluOpType.add,
                replica_groups=REPLICA_GROUPS,
                ins=[sR_in.opt()], outs=[sR_out.opt()],
            )

            # reduced S prefetch reuses the x_nat slots; [p, dt, d2-full]
            s_sb = b32_pool.tile([P, DT, D], BF16, tag="b32", name="s_sb")
            nc.sync.dma_start(s_sb[:, :, :FD],
                                sL_out.rearrange("p (t n) -> p t n", t=DT))
            nc.scalar.dma_start(s_sb[:, :, FD:],
                                sR_out.rearrange("p (t n) -> p t n", t=DT))

            # ---- query phase with UT/kv/out interleaved ----
            qt = {}

            def query_chunk(nch):
                qt_c = q8_pool.tile([P, ET, FD], BF16, tag="q8",
                                    name=f"qt{nch}")
                qt[nch] = qt_c
                for et in range(ET):
                    ps_r = ps_pool.tile([P, FD], F32, tag="ps", name="ps_qr")
                    for d in range(DT):
                        nc.tensor.matmul(
                            ps_r[:], wqr_sb[:, d, ts(et, P)],
                            xT[("r", nch)][:, d, :],
                            start=(d == 0), stop=(d == DT - 1),
                        )
                    nc.vector.tensor_copy(qt_c[:, et, :], ps_r[:])
                    ps_i = ps_pool.tile([P, FD], F32, tag="ps", name="ps_qi")
                    for d in range(DT):
                        nc.tensor.matmul(
                            ps_i[:], wqi_sb[:, d, ts(et, P)],
                            xT[("i", nch)][:, d, :],
                            start=(d == 0), stop=(d == DT - 1),
                        )
                    nc.vector.tensor_mul(
                        out=qt_c[:, et, :], in0=qt_c[:, et, :], in1=ps_i[:],
                    )

            # UT = S^T Wk-half: [d2-tile, k-half], two 4-bank passes;
            # pass lo=0 only needs the S-left AllReduce, so it slots in
            # right after query chunk 0
            ut_sb = ut_pool.tile([P, DT, KH], BF16, tag="ut", name="ut_sb")

            def ut_pass(lo):
                ps_u = [ps_pool.tile([P, KH], F32, tag="ps", name="ps_u")
                        for _ in range(4)]
                for i in range(4):
                    d2 = lo + i
                    for dt_ in range(DT):
                        nc.tensor.matmul(
                            ps_u[i][:], s_sb[:, dt_, ts(d2, P)],
                            wk_sb[:, dt_, :],
                            start=(dt_ == 0), stop=(dt_ == DT - 1),
                        )
                for i in range(4):
                    nc.vector.tensor_copy(ut_sb[:, lo + i, :], ps_u[i][:])

            query_chunk(0)
            ut_pass(0)
            query_chunk(1)
            ut_pass(4)

            wv_B = wkv_pool.tile([P, DT, FD], BF16, tag="wkv", name="wv_B")
            nc.scalar.dma_start(wv_B[:], as_tiles(wv)[:, :, FD:])

            # kv own-half rows: [own k-tile, e], 4 banks per e-half
            kv_view = kv_in.rearrange("p (h t n) -> p h t n", h=2, t=4)
            for eh, wv_sb in ((0, wv_A), (1, wv_B)):
                ps_k = [ps_pool.tile([P, FD], F32, tag="ps", name="ps_k")
                        for _ in range(4)]
                for kt in range(4):
                    for d2 in range(DT):
                        nc.tensor.matmul(
                            ps_k[kt][:], ut_sb[:, d2, ts(kt, P)],
                            wv_sb[:, d2, :],
                            start=(d2 == 0), stop=(d2 == DT - 1),
                        )
                kv_st = sst_pool.tile([P, 4, FD], BF16, tag="sst",
                                      name="kv_st")
                for kt in range(4):
                    nc.vector.tensor_copy(kv_st[:, kt, :], ps_k[kt][:])
                nc.gpsimd.dma_start(kv_view[:, eh], kv_st[:])

            nc.gpsimd.collective_compute(
                "AllGather", mybir.AluOpType.bypass,
                replica_groups=REPLICA_GROUPS,
                ins=[kv_in.opt()], outs=[kv_out.opt()],
            )
            # kv_sb split in two 8KB q8 tiles: [p, kt(4), e]
            kv_sb = []
            for r in range(2):
                kv_r = q8_pool.tile([P, 4, E], BF16, tag="q8", name=f"kv{r}")
                gview = kv_out[r].rearrange("p (h t n) -> p h t n", h=2, t=4)
                q = nc.sync.dma_start if r == 0 else nc.scalar.dma_start
                for eh in range(2):
                    q(kv_r[:, :, ts(eh, FD)], gview[:, eh])
                kv_sb.append(kv_r)

            query_chunk(2)
            query_chunk(3)

            # ---- out = queryT.T @ kv ----
            for nt in range(NT):
                nch, j = divmod(nt, 4)
                ps_o = [ps_pool.tile([P, FD], F32, tag="ps", name="ps_o")
                        for _ in range(2)]
                for et in range(ET):
                    lhsT = qt[nch][:, et, ts(j, P)]
                    for eh in range(2):
                        nc.tensor.matmul(
                            ps_o[eh][:], lhsT,
                            kv_sb[et // 4][:, et % 4, ts(eh, FD)],
                            start=(et == 0), stop=(et == ET - 1),
                        )
                for eh in range(2):
                    o_st = out_pool.tile([P, FD], F32, tag="ost", name="o_st")
                    nc.vector.tensor_copy(o_st[:], ps_o[eh][:])
                    q = nc.sync.dma_start if eh == 0 else nc.scalar.dma_start
                    q(out[ts(nt, P), ts(eh, FD)], o_st[:])

    nc.compile()
    return nc


def make_in_maps(x_real, x_imag, w_query_real, w_query_imag, w_key, w_value):
    ws = {
        "wqr": np.ascontiguousarray(np.asarray(w_query_real, np.float32)
                                    .astype(NP_BF16)),
        "wqi": np.ascontiguousarray(np.asarray(w_query_imag, np.float32)
                                    .astype(NP_BF16)),
        "wv": np.ascontiguousarray(np.asarray(w_value, np.float32)
                                   .astype(NP_BF16)),
    }
    wk = np.asarray(w_key, np.float32).astype(NP_BF16)
    xr_b = np.asarray(x_real, np.float32).astype(NP_BF16)
    xi_b = np.asarray(x_imag, np.float32).astype(NP_BF16)
    in_maps = []
    for c in range(N_CORES):
        b, h = divmod(c, 2)
        sl = slice(h * NL, (h + 1) * NL)
        in_maps.append({
            "xr": np.ascontiguousarray(xr_b[b, sl]),
            "xi": np.ascontiguousarray(xi_b[b, sl]),
            "wkh": np.ascontiguousarray(wk[:, h * KH:(h + 1) * KH]),
            **ws,
        })
    return in_maps


def gather_out(results):
    out = np.empty((B, N, E), np.float32)
    for c in range(N_CORES):
        b, h = divmod(c, 2)
        out[b, h * NL:(h + 1) * NL] = results[c]["out"]
    return out


def kernel(x_real, x_imag, w_query_real, w_query_imag, w_key, w_value):
    nc = build_bass()
    in_maps = make_in_maps(x_real, x_imag, w_query_real, w_query_imag,
                           w_key, w_value)
    res = run_bass_kernel_spmd(nc, in_maps, core_ids=list(range(N_CORES)))
    return gather_out(res.results)


if __name__ == "__main__":
    rng = np.random.default_rng(0)
    args = dict(
        x_real=rng.standard_normal((B, N, D), dtype=np.float32),
        x_imag=rng.standard_normal((B, N, D), dtype=np.float32),
        w_query_real=(rng.standard_normal((D, E), dtype=np.float32) / D),
        w_query_imag=(rng.standard_normal((D, E), dtype=np.float32) / D),
        w_key=(rng.standard_normal((D, E), dtype=np.float32) / D),
        w_value=(rng.standard_normal((D, E), dtype=np.float32) / D),
    )
    got = kernel(**args)
    q = np.einsum("bnd,de->bne", args["x_real"], args["w_query_real"]) * \
        np.einsum("bnd,de->bne", args["x_imag"], args["w_query_imag"])
    k = np.einsum("bnd,de->bne", args["x_real"], args["w_key"])
    v = np.einsum("bnd,de->bne", args["x_imag"], args["w_value"])
    kv = np.einsum("bnd,bne->bde", k, v)
    want = np.einsum("bnd,bde->bne", q, kv)
    denom = np.abs(want).max()
    print("max abs err:", np.abs(got - want).max())
    print("rel err:", np.abs(got - want).max() / denom)
